# revision 1
# baseline (speedup 1.0000x reference)
"""ConformerDecoder Trainium2 Bass kernel.

Sharding: pure data-parallel over batch B=8 -> one sample per NeuronCore.

All activations live in "transposed" layout [feature-on-partitions, T-free]
so the entire matmul chain (ffn1 -> qkv -> attn -> out-proj -> conv pw1 ->
pw2 -> ffn2) runs with weights as natural lhsT operands and zero activation
transposes.  bf16 matmuls with fp32 PSUM accumulation.

Host-side one-time model formatting: cast weights to bf16, fold constant
scales (0.25 into ffn w2 = silu-half * residual-half; 0.5 into pw2 =
conv-silu half; 0.5 into dw = GLU-sigmoid half; 1/sqrt(DH) into Wq), build
band masks / identity / ones as inline const tensors.

LayerNorm (feature axis = partitions): per-token sums via ones[128,128]
matmuls on PE (output rows are 128-replicated broadcasts for free), rsqrt
via seeded Newton on DVE.  silu/sigmoid via Tanh so a single ACT table set
(exp_and_others: Exp+Tanh) serves the whole kernel -- no table switches.

Windowed attention (W=64): per (head, q-block of 128): one scores matmul
against a 3-chunk (384-wide) zero-padded K window, full-tile Exp on ACT,
multiplicative band-mask scalar_tensor_tensor with accum_out giving the
softmax denominator, reciprocal + per-row scale, PE transposes of the 3
window chunks into one bf16 PSUM tile, 3 AV matmuls against natural-layout
zero-padded V chunks accumulating both heads of a pair into one PSUM tile.

Depthwise conv K=31: 31 fused scalar_tensor_tensor taps (acc = shift*w +
acc) on DVE; an odd-shifted copy of the padded GLU output keeps every tap
4-byte aligned (bf16 2x DVE mode).
"""

import os
import sys
from contextlib import ExitStack

for _p in ("/opt/trn_rl_repo",):
    if _p not in sys.path:
        sys.path.insert(0, _p)

import numpy as np
import ml_dtypes

import concourse.bass as bass
import concourse.tile as tile
from concourse import bacc
from concourse import mybir
from concourse.bass_utils import run_bass_kernel_spmd

BF16 = mybir.dt.bfloat16
F32 = mybir.dt.float32
AF = mybir.ActivationFunctionType
OP = mybir.AluOpType

L, D, H, T, B = 4, 512, 8, 1024, 8
FF = 4 * D            # 2048
EC = 2 * D            # 1024 conv channels
KK = 31               # conv kernel size
WIN = 64              # attention window
DH = D // H           # 64
P = 128
DC = D // P           # 4 feature chunks
FC = FF // P          # 16
CC = EC // P          # 8
TB = T // P           # 8 token blocks
NT = 512              # matmul moving free dim
TC = T // NT          # 2 t-columns
KW = 3 * P            # 384: attention window width
EPS = 1e-5

TRACE = False          # set by test.py for profiling runs
TRACE_KW = {}
LAST_RESULT = None     # BassKernelResults of last run (read by test.py)
LAYERS = int(os.environ.get("CONF_LAYERS", str(L)))
PHASES = os.environ.get("CONF_PHASES", "fac2b")

# bias row indices in the packed bias tensor
BR_F1B1, BR_F1B2, BR_Q, BR_K, BR_V, BR_O, BR_P1, BR_P2, BR_F2B1, BR_F2B2 = range(10)


def _band_masks():
    """[128, KW] multiplicative masks (mid, qb0, qb7) in bf16.

    Window column c for q-block qb is absolute key k = qb*128 - 128 + c;
    row i is query q = qb*128 + i.  Valid iff |q-k| <= WIN/2 and 0<=k<T.
    """
    i = np.arange(P)[:, None]
    c = np.arange(KW)[None, :]
    band = np.abs(i + P - c) <= WIN // 2
    q0 = band & (c >= P)       # k >= 0 when qb == 0
    q7 = band & (c <= 255)     # k <= 1023 when qb == 7
    to = lambda m: np.ascontiguousarray(m.astype(ml_dtypes.bfloat16))
    return to(band), to(q0), to(q7)


def build_program(flags):
    ln_gen = not flags["ln_trivial"]
    bias_gen = not flags["bias_trivial"]
    fin_gen = not flags["final_trivial"]
    dwb_gen = not flags["dwb_trivial"]

    nc = bacc.Bacc("TRN2", target_bir_lowering=False, debug=False)

    xt_d = nc.dram_tensor("x_t", [D, T], F32, kind="ExternalInput").ap()
    out_d = nc.dram_tensor("out_t", [D, T], F32, kind="ExternalOutput").ap()

    def win(name, shape):
        return nc.dram_tensor(name, shape, BF16, kind="ExternalInput").ap()

    w_f1a = win("f1w1", [L, D, FF])
    w_f1b = win("f1w2", [L, FF, D])
    w_f2a = win("f2w1", [L, D, FF])
    w_f2b = win("f2w2", [L, FF, D])
    w_q = win("wq", [L, D, D])
    w_kk = win("wk", [L, D, D])
    w_v = win("wv", [L, D, D])
    w_o = win("wo", [L, D, D])
    w_p1 = win("pw1", [L, D, 2 * EC])
    w_p2 = win("pw2", [L, EC, D])
    w_dw = nc.dram_tensor("dw", [L, P, CC, KK], F32, kind="ExternalInput").ap()
    w_gn = nc.dram_tensor("gn_aff", [L, 2, EC], F32, kind="ExternalInput").ap()
    w_gains = (nc.dram_tensor("ln_gains", [L, 10, D], F32, kind="ExternalInput").ap()
                if ln_gen else None)
    w_fin = (nc.dram_tensor("final_aff", [2, D], F32, kind="ExternalInput").ap()
             if fin_gen else None)
    w_bias = win("biases", [L, 10, 2 * EC]) if bias_gen else None
    w_dwb = (nc.dram_tensor("dwb", [L, P, CC], F32, kind="ExternalInput").ap()
             if dwb_gen else None)

    band_mid, band_q0, band_q7 = _band_masks()
    bmid_d = nc.inline_tensor(band_mid, "band_mid").ap()
    bq0_d = nc.inline_tensor(band_q0, "band_q0").ap()
    bq7_d = nc.inline_tensor(band_q7, "band_q7").ap()
    id_d = nc.inline_tensor(np.eye(P, dtype=ml_dtypes.bfloat16), "ident").ap()
    ones_d = nc.inline_tensor(np.ones((P, NT), dtype=ml_dtypes.bfloat16), "ones").ap()

    with tile.TileContext(nc) as tc, ExitStack() as ctx:
        pers = ctx.enter_context(tc.tile_pool(name="pers", bufs=1))
        wpool = ctx.enter_context(tc.tile_pool(name="w", bufs=1))
        spool = ctx.enter_context(tc.tile_pool(name="stat", bufs=1))
        hpool = ctx.enter_context(tc.tile_pool(name="h1", bufs=3))
        tpool = ctx.enter_context(tc.tile_pool(name="tanh", bufs=3))
        atpool = ctx.enter_context(tc.tile_pool(name="attn", bufs=3))
        smpool = ctx.enter_context(tc.tile_pool(name="small", bufs=8))
        cpool = ctx.enter_context(tc.tile_pool(name="conv", bufs=2))
        psum = ctx.enter_context(tc.tile_pool(name="ps", bufs=6, space="PSUM"))

        ident = pers.tile([P, P], BF16, tag="ident")
        nc.sync.dma_start(ident, id_d)
        ones = pers.tile([P, NT], BF16, tag="ones")
        nc.sync.dma_start(ones, ones_d)
        bands = {}
        for nm, dd in (("mid", bmid_d), ("q0", bq0_d), ("q7", bq7_d)):
            bt = pers.tile([P, KW], BF16, tag=f"band_{nm}")
            nc.sync.dma_start(bt, dd)
            bands[nm] = bt

        x = pers.tile([P, DC, T], BF16, tag="x")
        with tc.tile_pool(name="xin", bufs=2) as xinp:
            for kc in range(DC):
                xf = xinp.tile([P, T], F32, name=f"xf{kc}", tag="xf")
                nc.sync.dma_start(
                    xf, xt_d.rearrange("(c p) t -> c p t", p=P)[kc])
                nc.vector.tensor_copy(out=x[:, kc], in_=xf)

        xh = pers.tile([P, DC, T], BF16, tag="xhat")
        qT = pers.tile([P, DC, T], BF16, tag="qT")
        kT = pers.tile([P, DC, T + 2 * P], BF16, tag="kT")
        vN = pers.tile([P, TB + 2, D], BF16, tag="vN")
        oT = pers.tile([P, DC, T], BF16, tag="oT")
        c2 = pers.tile([P, CC, T], BF16, tag="c2")
        nc.vector.memset(kT[:, :, 0:P], 0.0)
        nc.vector.memset(kT[:, :, P + T :], 0.0)
        nc.vector.memset(vN[:, 0, :], 0.0)
        nc.vector.memset(vN[:, TB + 1, :], 0.0)

        def ln_rstd(var, out_r, niter=2):
            nc.vector.tensor_scalar(
                out=out_r, in0=var, scalar1=-0.5, scalar2=1.5,
                op0=OP.mult, op1=OP.add)
            t1 = spool.tile(list(var.shape), F32, tag="nt1")
            for _ in range(niter):
                nc.vector.tensor_tensor(t1, out_r, out_r, OP.mult)
                nc.vector.scalar_tensor_tensor(
                    out=t1, in0=t1, scalar=-0.5, in1=var, op0=OP.mult, op1=OP.mult)
                nc.vector.scalar_tensor_tensor(
                    out=out_r, in0=t1, scalar=1.5, in1=out_r, op0=OP.add, op1=OP.mult)

        def emit_ln(src, dst, lidx, which, out_stream=None, fin_sb=None):
            """LN over the feature (partition) axis of src -> dst, both
            [P, DC, T] bf16.  which selects the gain row pair."""
            x2 = spool.tile([P, CC, T], BF16, tag="x2")
            for kc in range(DC):
                nc.vector.tensor_tensor(x2[:, kc], src[:, kc], src[:, kc], OP.mult)
            r_bf = spool.tile([P, T], BF16, tag="r_bf")
            mr_bf = spool.tile([P, T], BF16, tag="mr_bf")
            for tci in range(TC):
                sl = slice(tci * NT, (tci + 1) * NT)
                ps_s = psum.tile([P, NT], F32, tag="mm")
                ps_q = psum.tile([P, NT], F32, tag="mm")
                for kc in range(DC):
                    nc.tensor.matmul(ps_s, lhsT=ones[:, 0:P], rhs=src[:, kc, sl],
                                     start=(kc == 0), stop=(kc == DC - 1))
                for kc in range(DC):
                    nc.tensor.matmul(ps_q, lhsT=ones[:, 0:P], rhs=x2[:, kc, sl],
                                     start=(kc == 0), stop=(kc == DC - 1))
                s_sb = spool.tile([P, NT], F32, tag="s_sb")
                nc.vector.tensor_copy(out=s_sb, in_=ps_s)
                msq = spool.tile([P, NT], F32, tag="msq")
                nc.vector.scalar_tensor_tensor(
                    out=msq, in0=s_sb, scalar=1.0 / (D * D), in1=s_sb,
                    op0=OP.mult, op1=OP.mult)
                var = spool.tile([P, NT], F32, tag="var")
                nc.vector.scalar_tensor_tensor(
                    out=var, in0=ps_q, scalar=1.0 / D, in1=msq,
                    op0=OP.mult, op1=OP.subtract)
                r = spool.tile([P, NT], F32, tag="r")
                ln_rstd(var, r)
                nc.vector.tensor_copy(out=r_bf[:, sl], in_=r)
                mr = spool.tile([P, NT], F32, tag="mr")
                nc.vector.scalar_tensor_tensor(
                    out=mr, in0=s_sb, scalar=1.0 / D, in1=r, op0=OP.mult, op1=OP.mult)
                nc.vector.tensor_copy(out=mr_bf[:, sl], in_=mr)
            g_sb = None
            if w_gains is not None:
                g_sb = spool.tile([P, 2, DC], F32, tag="g_sb")
                nc.sync.dma_start(
                    g_sb, w_gains[lidx, 2 * which : 2 * which + 2]
                    .rearrange("g (c p) -> p g c", p=P))
            for kc in range(DC):
                for tci in range(TC):
                    sl = slice(tci * NT, (tci + 1) * NT)
                    u = tpool.tile([P, NT], BF16, tag="ln_u")
                    nc.vector.tensor_tensor(u, src[:, kc, sl], r_bf[:, sl], OP.mult)
                    tgt = dst[:, kc, sl]
                    nc.vector.tensor_tensor(tgt, u, mr_bf[:, sl], OP.subtract)
                    if g_sb is not None:
                        nc.scalar.activation(
                            out=tgt, in_=tgt, func=AF.Identity,
                            bias=g_sb[:, 1, kc : kc + 1], scale=g_sb[:, 0, kc : kc + 1])
                    if out_stream is not None:
                        pool_o, dview = out_stream
                        of = pool_o.tile([P, NT], F32, tag="of")
                        nc.vector.tensor_tensor(of, u, mr_bf[:, sl], OP.subtract)
                        if g_sb is not None:
                            nc.scalar.activation(
                                out=of, in_=of, func=AF.Identity,
                                bias=g_sb[:, 1, kc : kc + 1],
                                scale=g_sb[:, 0, kc : kc + 1])
                        if fin_sb is not None:
                            nc.scalar.activation(
                                out=of, in_=of, func=AF.Identity,
                                bias=fin_sb[:, 1, kc : kc + 1],
                                scale=fin_sb[:, 0, kc : kc + 1])
                        nc.sync.dma_start(dview[:, kc, sl], of)

        def load_w(dram, lidx, tag):
            _, fin, fout = dram.shape
            wt = wpool.tile([P, fin // P, fout], BF16, tag=tag)
            nc.sync.dma_start(wt, dram[lidx].rearrange("(c p) f -> p c f", p=P))
            return wt

        bias_sb = [None]

        def bias_mm(ps, row, mslice, tcslice_n):
            """Add bias row (features mslice) into psum via a K=1 matmul."""
            if bias_sb[0] is None:
                return
            nc.tensor.matmul(
                ps, lhsT=bias_sb[0][0:1, row, mslice], rhs=ones[0:1, 0:tcslice_n],
                start=False, stop=True, skip_group_check=True)

        def emit_ffn(wa_d, wb_d, rows, lidx, src):
            w1 = load_w(wa_d, lidx, "w1")
            w2 = load_w(wb_d, lidx, "w2")
            for tci in range(TC):
                sl = slice(tci * NT, (tci + 1) * NT)
                acc = [psum.tile([P, NT], F32, tag="mm", name=f"acc{i}")
                       for i in range(DC)]
                for m in range(FC):
                    ph = psum.tile([P, NT], F32, tag="mm")
                    for kc in range(DC):
                        nc.tensor.matmul(
                            ph, lhsT=w1[:, kc, m * P : (m + 1) * P], rhs=src[:, kc, sl],
                            start=(kc == 0), stop=(kc == DC - 1 and not bias_gen))
                    bias_mm(ph, rows[0], slice(m * P, (m + 1) * P), NT)
                    hb = hpool.tile([P, NT], BF16, tag="h1")
                    nc.scalar.activation(out=hb, in_=ph, func=AF.Silu)
                    for dcc in range(DC):
                        nc.tensor.matmul(
                            acc[dcc], lhsT=w2[:, m, dcc * P : (dcc + 1) * P], rhs=hb,
                            start=(m == 0), stop=(m == FC - 1 and not bias_gen),
                            skip_group_check=True)
                for dcc in range(DC):
                    bias_mm(acc[dcc], rows[1], slice(dcc * P, (dcc + 1) * P), NT)
                    nc.vector.scalar_tensor_tensor(
                        out=x[:, dcc, sl], in0=acc[dcc], scalar=1.0,
                        in1=x[:, dcc, sl], op0=OP.bypass, op1=OP.add)

        for l in range(LAYERS):
            if bias_gen:
                bt = wpool.tile([1, 10, 2 * EC], BF16, tag="bias")
                nc.sync.dma_start(bt, w_bias[l])
                bias_sb[0] = bt

            # ===== FFN1 (half residual) =====
            if "f" in PHASES:
                if l == 0 or ln_gen:
                    emit_ln(x, xh, l, 0)
                    src1 = xh
                else:
                    src1 = x  # already unit-normalized by previous blk LN
                emit_ffn(w_f1a, w_f1b, (BR_F1B1, BR_F1B2), l, src1)

            # ===== local windowed MHSA =====
            if "a" in PHASES or "A" in PHASES:
                emit_ln(x, xh, l, 1)
                wq = load_w(w_q, l, "wq")
                wk = load_w(w_kk, l, "wk")
                wv = load_w(w_v, l, "wv")
                wo = load_w(w_o, l, "wo")
                for m in range(DC):
                    for tci in range(TC):
                        sl = slice(tci * NT, (tci + 1) * NT)
                        pq = psum.tile([P, NT], F32, tag="mm")
                        for kc in range(DC):
                            nc.tensor.matmul(
                                pq, lhsT=wq[:, kc, m * P : (m + 1) * P],
                                rhs=xh[:, kc, sl],
                                start=(kc == 0), stop=(kc == DC - 1 and not bias_gen))
                        bias_mm(pq, BR_Q, slice(m * P, (m + 1) * P), NT)
                        nc.scalar.copy(out=qT[:, m, sl], in_=pq)
                        pk = psum.tile([P, NT], F32, tag="mm")
                        for kc in range(DC):
                            nc.tensor.matmul(
                                pk, lhsT=wk[:, kc, m * P : (m + 1) * P],
                                rhs=xh[:, kc, sl],
                                start=(kc == 0), stop=(kc == DC - 1 and not bias_gen))
                        bias_mm(pk, BR_K, slice(m * P, (m + 1) * P), NT)
                        nc.vector.tensor_copy(
                            out=kT[:, m, P + tci * NT : P + (tci + 1) * NT], in_=pk)
                for tb in range(TB):
                    pv = psum.tile([P, NT], F32, tag="mm")
                    for kc in range(DC):
                        nc.tensor.matmul(
                            pv, lhsT=xh[:, kc, tb * P : (tb + 1) * P],
                            rhs=wv[:, kc, 0:D],
                            start=(kc == 0), stop=(kc == DC - 1 and not bias_gen))
                    if bias_gen:
                        nc.tensor.matmul(
                            pv, lhsT=ones[0:1, 0:P], rhs=bias_sb[0][0:1, BR_V, 0:D],
                            start=False, stop=True, skip_group_check=True)
                    nc.vector.tensor_copy(out=vN[:, tb + 1, :], in_=pv)
                for hp in range(DC):
                    for qb in range(TB):
                        band = bands["q0"] if qb == 0 else (
                            bands["q7"] if qb == TB - 1 else bands["mid"])
                        po = psum.tile([P, P], F32, tag="mm")
                        for hh in range(2):
                            pr = slice(hh * DH, (hh + 1) * DH)
                            ps_s = psum.tile([P, KW], F32, tag="mm")
                            nc.tensor.matmul(
                                ps_s, lhsT=qT[pr, hp, qb * P : (qb + 1) * P],
                                rhs=kT[pr, hp, qb * P : qb * P + KW],
                                start=True, stop=True)
                            at = atpool.tile([P, KW], BF16, tag="at")
                            nc.scalar.activation(out=at, in_=ps_s, func=AF.Exp)
                            lsum = smpool.tile([P, 1], F32, tag="l")
                            nc.vector.scalar_tensor_tensor(
                                out=at, in0=at, scalar=1.0, in1=band,
                                op0=OP.bypass, op1=OP.mult, accum_out=lsum)
                            rl = smpool.tile([P, 1], F32, tag="rl")
                            nc.vector.reciprocal(out=rl, in_=lsum)
                            nc.vector.tensor_scalar_mul(out=at, in0=at, scalar1=rl)
                            pt = psum.tile([P, 224], BF16, tag="mm")
                            nc.tensor.transpose(pt[:, 0:32], at[0:32, 0:P],
                                                ident[0:32, 0:32])
                            nc.tensor.transpose(pt[:, 32:160], at[:, P : 2 * P], ident)
                            nc.tensor.transpose(pt[:, 160:224], at[64:128, 2 * P :],
                                                ident[64:128, 64:128])
                            asb = atpool.tile([P, 224], BF16, tag="asb")
                            nc.vector.tensor_copy(out=asb, in_=pt)
                            h = hp * 2 + hh
                            hc = slice(h * DH, (h + 1) * DH)
                            nc.tensor.matmul(
                                po[pr, :], lhsT=vN[:, qb + 1, hc], rhs=asb[:, 32:160],
                                start=True, stop=False, skip_group_check=True)
                            nc.tensor.matmul(
                                po[pr, 0:32], lhsT=vN[:, qb, hc], rhs=asb[:, 0:32],
                                start=False, stop=False, skip_group_check=True)
                            nc.tensor.matmul(
                                po[pr, 64:128], lhsT=vN[:, qb + 2, hc],
                                rhs=asb[:, 160:224],
                                start=False, stop=True, skip_group_check=True)
                        nc.vector.tensor_copy(
                            out=oT[:, hp, qb * P : (qb + 1) * P], in_=po)
                if "A" not in PHASES:
                    for tci in range(TC):
                        sl = slice(tci * NT, (tci + 1) * NT)
                        for m in range(DC):
                            pp = psum.tile([P, NT], F32, tag="mm")
                            for kc in range(DC):
                                nc.tensor.matmul(
                                    pp, lhsT=wo[:, kc, m * P : (m + 1) * P],
                                    rhs=oT[:, kc, sl],
                                    start=(kc == 0), stop=(kc == DC - 1 and not bias_gen))
                            bias_mm(pp, BR_O, slice(m * P, (m + 1) * P), NT)
                            nc.vector.scalar_tensor_tensor(
                                out=x[:, m, sl], in0=pp, scalar=1.0, in1=x[:, m, sl],
                                op0=OP.bypass, op1=OP.add)

            # ===== convolution module =====
            if "c" in PHASES:
                emit_ln(x, xh, l, 2)
                p1 = load_w(w_p1, l, "w1")
                p2 = load_w(w_p2, l, "w2")
                dwt = wpool.tile([P, CC, KK], F32, tag="dw")
                nc.sync.dma_start(dwt, w_dw[l])
                dwb_sb = None
                if dwb_gen:
                    dwb_sb = wpool.tile([P, CC], F32, tag="dwb")
                    nc.sync.dma_start(dwb_sb, w_dwb[l])
                for m in range(CC):
                    cp = cpool.tile([P, KK - 1 + T + 1], BF16, tag="cp")
                    co = cpool.tile([P, KK - 1 + T + 1], BF16, tag="co")
                    nc.vector.memset(cp[:, 0 : KK // 2], 0.0)
                    nc.vector.memset(cp[:, KK // 2 + T :], 0.0)
                    for tci in range(TC):
                        sl = slice(tci * NT, (tci + 1) * NT)
                        pb = psum.tile([P, NT], F32, tag="mm")
                        for kc in range(DC):
                            nc.tensor.matmul(
                                pb, lhsT=p1[:, kc, EC + m * P : EC + (m + 1) * P],
                                rhs=xh[:, kc, sl],
                                start=(kc == 0), stop=(kc == DC - 1 and not bias_gen))
                        bias_mm(pb, BR_P1, slice(EC + m * P, EC + (m + 1) * P), NT)
                        tb_ = tpool.tile([P, NT], BF16, tag="th")
                        nc.scalar.activation(out=tb_, in_=pb, func=AF.Tanh, scale=0.5)
                        pa = psum.tile([P, NT], F32, tag="mm")
                        for kc in range(DC):
                            nc.tensor.matmul(
                                pa, lhsT=p1[:, kc, m * P : (m + 1) * P],
                                rhs=xh[:, kc, sl],
                                start=(kc == 0), stop=(kc == DC - 1 and not bias_gen))
                        bias_mm(pa, BR_P1, slice(m * P, (m + 1) * P), NT)
                        nc.vector.scalar_tensor_tensor(
                            out=cp[:, KK // 2 + tci * NT : KK // 2 + (tci + 1) * NT],
                            in0=tb_, scalar=1.0, in1=pa, op0=OP.add, op1=OP.mult)
                    nc.vector.tensor_copy(out=co[:, 0 : KK - 1 + T],
                                          in_=cp[:, 1 : KK + T])
                    strip = cpool.tile([P, KK, P], BF16, tag="strip")
                    for kk in range(KK):
                        nc.vector.tensor_scalar_mul(
                            out=strip[:, kk, :], in0=ident,
                            scalar1=dwt[:, m, kk : kk + 1])
                    for tci in range(TC):
                        pc = psum.tile([P, NT], F32, tag="mm")
                        for kk in range(KK):
                            rhs = (cp[:, kk + tci * NT : kk + tci * NT + NT]
                                   if kk % 2 == 0 else
                                   co[:, kk - 1 + tci * NT : kk - 1 + tci * NT + NT])
                            nc.tensor.matmul(pc, lhsT=strip[:, kk, :], rhs=rhs,
                                             start=(kk == 0), stop=(kk == KK - 1),
                                             skip_group_check=True)
                        csl = c2[:, m, tci * NT : (tci + 1) * NT]
                        if dwb_sb is not None:
                            nc.vector.tensor_scalar_add(out=csl, in0=pc,
                                                        scalar1=dwb_sb[:, m : m + 1])
                        else:
                            nc.vector.tensor_copy(out=csl, in_=pc)
                # GroupNorm(1 group over [EC, T]) + silu fused
                cs = spool.tile([P, CC, T], BF16, tag="x2")
                for m in range(CC):
                    nc.vector.tensor_tensor(cs[:, m], c2[:, m], c2[:, m], OP.mult)
                parts = []
                for tci in range(TC):
                    sl = slice(tci * NT, (tci + 1) * NT)
                    ps_s = psum.tile([P, NT], F32, tag="mm")
                    ps_q = psum.tile([P, NT], F32, tag="mm")
                    for m in range(CC):
                        nc.tensor.matmul(ps_s, lhsT=ones[:, 0:P], rhs=c2[:, m, sl],
                                         start=(m == 0), stop=(m == CC - 1))
                    for m in range(CC):
                        nc.tensor.matmul(ps_q, lhsT=ones[:, 0:P], rhs=cs[:, m, sl],
                                         start=(m == 0), stop=(m == CC - 1))
                    rs = smpool.tile([P, 1], F32, tag=f"gs{tci}")
                    rq = smpool.tile([P, 1], F32, tag=f"gq{tci}")
                    nc.vector.tensor_reduce(out=rs, in_=ps_s,
                                            axis=mybir.AxisListType.X, op=OP.add)
                    nc.vector.tensor_reduce(out=rq, in_=ps_q,
                                            axis=mybir.AxisListType.X, op=OP.add)
                    parts.append((rs, rq))
                gs = smpool.tile([P, 1], F32, tag="gsum")
                gq = smpool.tile([P, 1], F32, tag="gqsum")
                nc.vector.tensor_tensor(gs, parts[0][0], parts[1][0], OP.add)
                nc.vector.tensor_tensor(gq, parts[0][1], parts[1][1], OP.add)
                mg = smpool.tile([P, 1], F32, tag="mg")
                nc.vector.tensor_scalar_mul(out=mg, in0=gs, scalar1=1.0 / (EC * T))
                msqg = smpool.tile([P, 1], F32, tag="msqg")
                nc.vector.tensor_tensor(msqg, mg, mg, OP.mult)
                varg = smpool.tile([P, 1], F32, tag="varg")
                nc.vector.scalar_tensor_tensor(
                    out=varg, in0=gq, scalar=1.0 / (EC * T), in1=msqg,
                    op0=OP.mult, op1=OP.subtract)
                nc.vector.tensor_scalar_add(out=varg, in0=varg, scalar1=EPS)
                rg = smpool.tile([P, 1], F32, tag="rg")
                ln_rstd(varg, rg, niter=14)
                # A = gn_g * r ; B = gn_b - m * A    (per-channel, [P, CC])
                gaff = spool.tile([P, 2, CC], F32, tag="gaff")
                nc.sync.dma_start(gaff, w_gn[l].rearrange("g (c p) -> p g c", p=P))
                a_t = spool.tile([P, CC], F32, tag="a_t")
                nc.vector.tensor_scalar_mul(out=a_t, in0=gaff[:, 0], scalar1=rg)
                mneg = smpool.tile([P, 1], F32, tag="mneg")
                nc.vector.tensor_scalar_mul(out=mneg, in0=mg, scalar1=-1.0)
                b_t = spool.tile([P, CC], F32, tag="b_t")
                nc.vector.scalar_tensor_tensor(
                    out=b_t, in0=a_t, scalar=mneg, in1=gaff[:, 1],
                    op0=OP.mult, op1=OP.add)
                b_bf = spool.tile([P, CC], BF16, tag="b_bf")
                nc.vector.tensor_copy(out=b_bf, in_=b_t)
                for m in range(CC):
                    y2 = cpool.tile([P, T], BF16, tag="y2")
                    nc.vector.scalar_tensor_tensor(
                        out=y2, in0=c2[:, m], scalar=a_t[:, m : m + 1],
                        in1=b_bf[:, m : m + 1].to_broadcast((P, T)),
                        op0=OP.mult, op1=OP.add)
                    nc.scalar.activation(out=c2[:, m], in_=y2, func=AF.Silu)
                for tci in range(TC):
                    sl = slice(tci * NT, (tci + 1) * NT)
                    for dcc in range(DC):
                        pp = psum.tile([P, NT], F32, tag="mm")
                        for m in range(CC):
                            nc.tensor.matmul(
                                pp, lhsT=p2[:, m, dcc * P : (dcc + 1) * P],
                                rhs=c2[:, m, sl],
                                start=(m == 0), stop=(m == CC - 1 and not bias_gen))
                        bias_mm(pp, BR_P2, slice(dcc * P, (dcc + 1) * P), NT)
                        nc.vector.scalar_tensor_tensor(
                            out=x[:, dcc, sl], in0=pp, scalar=1.0, in1=x[:, dcc, sl],
                            op0=OP.bypass, op1=OP.add)

            # ===== FFN2 (half residual) =====
            if "2" in PHASES:
                emit_ln(x, xh, l, 3)
                emit_ffn(w_f2a, w_f2b, (BR_F2B1, BR_F2B2), l, xh)

            # ===== per-block LN =====
            if "b" in PHASES:
                if l == LAYERS - 1:
                    with tc.tile_pool(name="outp", bufs=3) as op_:
                        fin_sb = None
                        if w_fin is not None:
                            fin_sb = spool.tile([P, 2, DC], F32, tag="fin_sb")
                            nc.sync.dma_start(
                                fin_sb, w_fin.rearrange("g (c p) -> p g c", p=P))
                        emit_ln(x, x, l, 4,
                                out_stream=(op_, out_d.rearrange(
                                    "(c p) t -> p c t", p=P)),
                                fin_sb=fin_sb)
                else:
                    emit_ln(x, x, l, 4)

        if "b" not in PHASES or LAYERS == 0:
            # debug path: dump current x (or oT for 'A') as output
            with tc.tile_pool(name="outp", bufs=3) as op_:
                srcd = oT if "A" in PHASES else x
                dview = out_d.rearrange("(c p) t -> p c t", p=P)
                for kc in range(DC):
                    for tci in range(TC):
                        sl = slice(tci * NT, (tci + 1) * NT)
                        of = op_.tile([P, NT], F32, tag="of")
                        nc.vector.tensor_copy(out=of, in_=srcd[:, kc, sl])
                        nc.sync.dma_start(dview[:, kc, sl], of)

    nc.finalize()
    return nc


_PROG_CACHE = {}


def _get_program(flags):
    key = tuple(sorted(flags.items())) + (LAYERS, PHASES)
    if key not in _PROG_CACHE:
        _PROG_CACHE[key] = build_program(flags)
    return _PROG_CACHE[key]


def kernel(**inputs):
    global LAST_RESULT
    f32 = lambda a: np.asarray(a, dtype=np.float32)
    bf = lambda a: np.ascontiguousarray(f32(a).astype(ml_dtypes.bfloat16))
    x = f32(inputs["x"])                       # [B, T, D]

    def triv(names_vals):
        return all(bool(np.all(f32(inputs[n]) == v)) for n, v in names_vals)

    ln_trivial = triv(
        [(f"{p}_ln_g", 1.0) for p in ("ffn1", "attn", "conv", "ffn2", "blk")]
        + [(f"{p}_ln_b", 0.0) for p in ("ffn1", "attn", "conv", "ffn2", "blk")])
    final_trivial = triv([("final_ln_g", 1.0), ("final_ln_b", 0.0)])
    bias_trivial = triv([(n, 0.0) for n in (
        "ffn1_b1", "ffn1_b2", "qkv_b", "outp_b", "pw1_b", "pw2_b",
        "ffn2_b1", "ffn2_b2")])
    dwb_trivial = triv([("dw_b", 0.0)])
    flags = dict(ln_trivial=ln_trivial, final_trivial=final_trivial,
                 bias_trivial=bias_trivial, dwb_trivial=dwb_trivial)

    nc = _get_program(flags)

    qkv = f32(inputs["qkv_w"])                # [L, D, 3D]
    dw = f32(inputs["dw_w"]).reshape(L, EC, KK) * 0.5
    dw = dw.reshape(L, CC, P, KK).transpose(0, 2, 1, 3)  # [L, P, CC, K]
    gn_aff = np.stack([f32(inputs["gn_g"]), f32(inputs["gn_b"])], axis=1)

    common = {
        "f1w1": bf(inputs["ffn1_w1"]),
        "f1w2": bf(f32(inputs["ffn1_w2"]) * 0.5),
        "f2w1": bf(inputs["ffn2_w1"]),
        "f2w2": bf(f32(inputs["ffn2_w2"]) * 0.5),
        "wq": bf(qkv[:, :, 0:D] * (DH ** -0.5)),
        "wk": bf(qkv[:, :, D : 2 * D]),
        "wv": bf(qkv[:, :, 2 * D : 3 * D]),
        "wo": bf(inputs["outp_w"]),
        "pw1": bf(inputs["pw1_w"]),
        "pw2": bf(inputs["pw2_w"]),
        "dw": np.ascontiguousarray(dw.astype(np.float32)),
        "gn_aff": np.ascontiguousarray(gn_aff.astype(np.float32)),
    }
    if not ln_trivial:
        rows = []
        for pfx in ("ffn1", "attn", "conv", "ffn2", "blk"):
            rows.append(f32(inputs[f"{pfx}_ln_g"]))
            rows.append(f32(inputs[f"{pfx}_ln_b"]))
        common["ln_gains"] = np.ascontiguousarray(
            np.stack(rows, axis=1).astype(np.float32))  # [L, 10, D]
    if not final_trivial:
        common["final_aff"] = np.ascontiguousarray(np.stack(
            [f32(inputs["final_ln_g"]), f32(inputs["final_ln_b"])]).astype(np.float32))
    if not bias_trivial:
        bias = np.zeros((L, 10, 2 * EC), np.float32)
        qb = f32(inputs["qkv_b"])
        bias[:, BR_F1B1, :FF] = f32(inputs["ffn1_b1"])
        bias[:, BR_F1B2, :D] = f32(inputs["ffn1_b2"]) * 0.5
        bias[:, BR_Q, :D] = qb[:, 0:D] * (DH ** -0.5)
        bias[:, BR_K, :D] = qb[:, D : 2 * D]
        bias[:, BR_V, :D] = qb[:, 2 * D : 3 * D]
        bias[:, BR_O, :D] = f32(inputs["outp_b"])
        bias[:, BR_P1, : 2 * EC] = f32(inputs["pw1_b"])
        bias[:, BR_P2, :D] = f32(inputs["pw2_b"])
        bias[:, BR_F2B1, :FF] = f32(inputs["ffn2_b1"])
        bias[:, BR_F2B2, :D] = f32(inputs["ffn2_b2"]) * 0.5
        common["biases"] = bf(bias)
    if not dwb_trivial:
        dwb = f32(inputs["dw_b"]).reshape(L, CC, P).transpose(0, 2, 1)
        common["dwb"] = np.ascontiguousarray(dwb.astype(np.float32))

    in_maps = []
    for c in range(B):
        m = dict(common)
        m["x_t"] = np.ascontiguousarray(x[c].T)   # [D, T] fp32
        in_maps.append(m)

    res = run_bass_kernel_spmd(
        nc, in_maps, core_ids=list(range(B)), trace=TRACE, **TRACE_KW)
    LAST_RESULT = res
    out = np.stack([r["out_t"].T for r in res.results]).astype(np.float32)
    return out


if __name__ == "__main__":
    rng = np.random.default_rng(0)
    ins = {"x": rng.standard_normal((B, T, D), dtype=np.float32)}
    # minimal smoke test requires full inputs; use test.py instead
    print("use test.py")



# revision 6
# speedup vs baseline: 1.3181x; 1.3181x over previous
"""ConformerDecoder Trainium2 Bass kernel (stripe-pipelined).

Sharding: pure data-parallel over batch B=8 -> one sample per NeuronCore.

Activations live transposed [feature-on-partitions, T-free]; all matmuls use
natural-layout bf16 weights as lhsT with fp32 PSUM accumulation.

Key structure vs the v1 kernel:
- Every LayerNorm is stripe-pipelined: T is split in two 512-token stripes;
  a phase's stripe-1 matmuls run on PE while stripe-0's LN stats (ACT squares
  -> PE ones-matmul sums -> DVE var/Newton-rsqrt chain) and normalize run on
  ACT/DVE.  Stat matmuls are injected mid-loop into the neighboring matmul
  stretch so PE never drains.
- Attention computes scores TRANSPOSED (k-on-partitions) with 3 chunk
  matmuls per (head-pair, q-block, head): exp on ACT, band-mask multiply on
  DVE, then AV + softmax-denominator ones-matmuls accumulate straight off the
  masked exp tile -- no PE transposes, no per-row softmax scaling; one
  reciprocal + one scaled PSUM evacuation per (head-pair, q-block).
- Conv GroupNorm sum/sumsq matmuls accumulate inside the depthwise-conv loop;
  GN affine + SiLU fuse into a single ACT op per channel chunk feeding
  m-outer pw2 accumulation.
- DVE relief: squares and PSUM evacuations on ACT, GLU via Tanh, dw-conv
  diag strips built with one broadcast-view multiply per chunk.
"""

import os
import sys
from contextlib import ExitStack

for _p in ("/opt/trn_rl_repo",):
    if _p not in sys.path:
        sys.path.insert(0, _p)

import numpy as np
import ml_dtypes

import concourse.bass as bass
import concourse.tile as tile
from concourse import bacc
from concourse import mybir
from concourse.bass_utils import run_bass_kernel_spmd

BF16 = mybir.dt.bfloat16
F32 = mybir.dt.float32
AF = mybir.ActivationFunctionType
OP = mybir.AluOpType

L, D, H, T, B = 4, 512, 8, 1024, 8
FF = 4 * D            # 2048
EC = 2 * D            # 1024 conv channels
KK = 31               # conv kernel size
WIN = 64              # attention window
DH = D // H           # 64
P = 128
DC = D // P           # 4 feature chunks
FC = FF // P          # 16
CC = EC // P          # 8
TB = T // P           # 8 token blocks
NT = 512              # matmul moving free dim
TC = T // NT          # 2 t-stripes
KW = 3 * P            # 384: attention window width
EPS = 1e-5

TRACE = False          # set by test.py for profiling runs
TRACE_KW = {}
LAST_RESULT = None     # BassKernelResults of last run (read by test.py)
LAYERS = int(os.environ.get("CONF_LAYERS", str(L)))

# bias row indices in the packed bias tensor
BR_F1B1, BR_F1B2, BR_Q, BR_K, BR_V, BR_O, BR_P1, BR_P2, BR_F2B1, BR_F2B2 = range(10)


def _band_masks_t():
    """Transposed band masks [P, 3, P] bf16: maskT[k, c, q] = valid(q, c*128+k).

    Window column c128+k for q-block qb is absolute key kk = qb*128 - 128 +
    (c*128+k); row q is query qb*128 + q.  Valid iff |q-kk| <= WIN/2 and
    0 <= kk < T.
    """
    q = np.arange(P)[:, None]
    c = np.arange(KW)[None, :]
    band = np.abs(q + P - c) <= WIN // 2          # [q, kw]
    q0 = band & (c >= P)
    q7 = band & (c <= 255)

    def to(m):
        mt = m.T.reshape(3, P, P).transpose(1, 0, 2)   # [k, c, q]
        return np.ascontiguousarray(mt.astype(ml_dtypes.bfloat16))

    return to(band), to(q0), to(q7)


def build_program(flags):
    ln_gen = not flags["ln_trivial"]
    bias_gen = not flags["bias_trivial"]
    fin_gen = not flags["final_trivial"]
    dwb_gen = not flags["dwb_trivial"]

    nc = bacc.Bacc("TRN2", target_bir_lowering=False, debug=False)

    xt_d = nc.dram_tensor("x_t", [D, T], F32, kind="ExternalInput").ap()
    out_d = nc.dram_tensor("out_t", [D, T], F32, kind="ExternalOutput").ap()

    def win(name, shape):
        return nc.dram_tensor(name, shape, BF16, kind="ExternalInput").ap()

    w_f1a = win("f1w1", [L, D, FF])
    w_f1b = win("f1w2", [L, FF, D])
    w_f2a = win("f2w1", [L, D, FF])
    w_f2b = win("f2w2", [L, FF, D])
    w_q = win("wq", [L, D, D])
    w_kk = win("wk", [L, D, D])
    w_v = win("wv", [L, D, D])
    w_o = win("wo", [L, D, D])
    w_p1 = win("pw1", [L, D, 2 * EC])
    w_p2 = win("pw2", [L, EC, D])
    w_dw = nc.dram_tensor("dw", [L, P, CC, KK], F32, kind="ExternalInput").ap()
    w_gn = nc.dram_tensor("gn_aff", [L, 2, EC], F32, kind="ExternalInput").ap()
    w_gains = (nc.dram_tensor("ln_gains", [L, 10, D], F32, kind="ExternalInput").ap()
                if ln_gen else None)
    w_fin = (nc.dram_tensor("final_aff", [2, D], F32, kind="ExternalInput").ap()
             if fin_gen else None)
    w_bias = win("biases", [L, 10, 2 * EC]) if bias_gen else None
    w_dwb = (nc.dram_tensor("dwb", [L, P, CC], F32, kind="ExternalInput").ap()
             if dwb_gen else None)

    mt_mid, mt_q0, mt_q7 = _band_masks_t()
    bmid_d = nc.inline_tensor(mt_mid, "bandt_mid").ap()
    bq0_d = nc.inline_tensor(mt_q0, "bandt_q0").ap()
    bq7_d = nc.inline_tensor(mt_q7, "bandt_q7").ap()
    id_d = nc.inline_tensor(np.eye(P, dtype=ml_dtypes.bfloat16), "ident").ap()
    ones_d = nc.inline_tensor(np.ones((P, NT), dtype=ml_dtypes.bfloat16), "ones").ap()

    with tile.TileContext(nc) as tc, ExitStack() as ctx:
        pers = ctx.enter_context(tc.tile_pool(name="pers", bufs=1))
        wpool = ctx.enter_context(tc.tile_pool(name="w", bufs=1))
        spool = ctx.enter_context(tc.tile_pool(name="stat", bufs=1))
        hpool = ctx.enter_context(tc.tile_pool(name="h1", bufs=3))
        tpool = ctx.enter_context(tc.tile_pool(name="tanh", bufs=3))
        atpool = ctx.enter_context(tc.tile_pool(name="attn", bufs=1))
        smpool = ctx.enter_context(tc.tile_pool(name="small", bufs=8))
        cpool = ctx.enter_context(tc.tile_pool(name="conv", bufs=1))
        opool = ctx.enter_context(tc.tile_pool(name="outp", bufs=2))
        psum = ctx.enter_context(tc.tile_pool(name="ps", bufs=8, space="PSUM"))

        ident = pers.tile([P, P], BF16, tag="ident")
        nc.sync.dma_start(ident, id_d)
        ones = pers.tile([P, NT], BF16, tag="ones")
        nc.sync.dma_start(ones, ones_d)
        bands = {}
        for nm, dd in (("mid", bmid_d), ("q0", bq0_d), ("q7", bq7_d)):
            bt = pers.tile([P, 3, P], BF16, tag=f"band_{nm}")
            nc.sync.dma_start(bt, dd)
            bands[nm] = bt

        x = pers.tile([P, DC, T], BF16, tag="x")
        with tc.tile_pool(name="xin", bufs=2) as xinp:
            for kc in range(DC):
                xf = xinp.tile([P, T], F32, name=f"xf{kc}", tag="xf")
                nc.sync.dma_start(
                    xf, xt_d.rearrange("(c p) t -> c p t", p=P)[kc])
                nc.scalar.copy(out=x[:, kc], in_=xf)

        xh = pers.tile([P, DC, T], BF16, tag="xhat")
        qT = pers.tile([P, DC, T], BF16, tag="qT")
        kT = pers.tile([P, DC, T + 2 * P], BF16, tag="kT")
        vN = pers.tile([P, TB + 2, D], BF16, tag="vN")
        oT = pers.tile([P, DC, T], BF16, tag="oT")
        c2 = pers.tile([P, CC, T], BF16, tag="c2")
        nc.vector.memset(kT[:, :, 0:P], 0.0)
        nc.vector.memset(kT[:, :, P + T :], 0.0)
        nc.vector.memset(vN[:, 0, :], 0.0)
        nc.vector.memset(vN[:, TB + 1, :], 0.0)

        # ---------------- LayerNorm (stripe-pipelined) ----------------
        # state per LN instance: x2, psum pair, r_bf/mr_bf per stripe

        def ln_rstd(var, out_r, niter=2, final_out=None):
            r = spool.tile(list(var.shape), F32, tag="newt_r", bufs=2)
            nc.vector.tensor_scalar(
                out=r, in0=var, scalar1=-0.5, scalar2=1.5,
                op0=OP.mult, op1=OP.add)
            t1 = spool.tile(list(var.shape), F32, tag="newt_t", bufs=2)
            for it in range(niter):
                nc.vector.tensor_tensor(t1, r, r, OP.mult)
                nc.vector.scalar_tensor_tensor(
                    out=t1, in0=t1, scalar=-0.5, in1=var, op0=OP.mult, op1=OP.mult)
                dst = out_r if it == niter - 1 else r
                nc.vector.scalar_tensor_tensor(
                    out=dst, in0=t1, scalar=1.5, in1=r, op0=OP.add, op1=OP.mult)

        class LNState:
            __slots__ = ("src", "lidx", "which", "ps", "r_bf", "mr_bf", "x2")

        def ln_sq(src, lidx, which, tci, st=None):
            """ACT squares for stripe tci; allocates state on first call."""
            if st is None:
                st = LNState()
                st.src, st.lidx, st.which = src, lidx, which
                st.ps = [None, None]
                st.r_bf = spool.tile([P, T], BF16, tag="r_bf", bufs=3)
                st.mr_bf = spool.tile([P, T], BF16, tag="mr_bf", bufs=3)
                st.x2 = [None, None]
            sl = slice(tci * NT, (tci + 1) * NT)
            x2 = spool.tile([P, DC, NT], BF16, tag="ln_x2", bufs=1)
            nc.scalar.activation(out=x2, in_=st.src[:, :, sl], func=AF.Square)
            st.x2[tci] = x2
            return st

        def ln_sums(st, tci):
            """PE ones-matmul sums for stripe tci (inject into a mm stretch)."""
            sl = slice(tci * NT, (tci + 1) * NT)
            ps_s = psum.tile([P, NT], F32, tag="mm", name="ps_s")
            ps_q = psum.tile([P, NT], F32, tag="mm", name="ps_q")
            for kc in range(DC):
                nc.tensor.matmul(ps_s, lhsT=ones[:, 0:P], rhs=st.src[:, kc, sl],
                                 start=(kc == 0), stop=(kc == DC - 1))
            for kc in range(DC):
                nc.tensor.matmul(ps_q, lhsT=ones[:, 0:P], rhs=st.x2[tci][:, kc],
                                 start=(kc == 0), stop=(kc == DC - 1))
            st.ps[tci] = (ps_s, ps_q)

        def ln_var(st, tci):
            """msq/mean on ACT, var + Newton rsqrt + mr on DVE."""
            sl = slice(tci * NT, (tci + 1) * NT)
            ps_s, ps_q = st.ps[tci]
            msq = spool.tile([P, NT], F32, tag="msq", bufs=1)
            nc.scalar.activation(out=msq, in_=ps_s, func=AF.Square, scale=1.0 / D)
            mean = spool.tile([P, NT], F32, tag="mean", bufs=1)
            nc.scalar.mul(out=mean, in_=ps_s, mul=1.0 / D)
            var = spool.tile([P, NT], F32, tag="var", bufs=2)
            nc.vector.scalar_tensor_tensor(
                out=var, in0=ps_q, scalar=1.0 / D, in1=msq,
                op0=OP.mult, op1=OP.subtract)
            ln_rstd(var, st.r_bf[:, sl])
            nc.vector.tensor_tensor(st.mr_bf[:, sl], mean, st.r_bf[:, sl], OP.mult)

        def ln_norm(st, tci, dst, out_stream=None, fin_sb=None):
            """normalize stripe tci of src into dst (DVE)."""
            sl = slice(tci * NT, (tci + 1) * NT)
            g_sb = None
            if w_gains is not None:
                g_sb = spool.tile([P, 2, DC], F32, tag="g_sb", bufs=2)
                nc.sync.dma_start(
                    g_sb, w_gains[st.lidx, 2 * st.which : 2 * st.which + 2]
                    .rearrange("g (c p) -> p g c", p=P))
            for kc in range(DC):
                u = tpool.tile([P, NT], BF16, tag="ln_u")
                nc.vector.tensor_tensor(u, st.src[:, kc, sl], st.r_bf[:, sl],
                                        OP.mult)
                if dst is not None:
                    tgt = dst[:, kc, sl]
                    nc.vector.tensor_tensor(tgt, u, st.mr_bf[:, sl], OP.subtract)
                    if g_sb is not None:
                        nc.scalar.activation(
                            out=tgt, in_=tgt, func=AF.Identity,
                            bias=g_sb[:, 1, kc : kc + 1], scale=g_sb[:, 0, kc : kc + 1])
                if out_stream is not None:
                    dview = out_stream
                    of = opool.tile([P, NT], F32, tag="of")
                    nc.vector.tensor_tensor(of, u, st.mr_bf[:, sl], OP.subtract)
                    if g_sb is not None:
                        nc.scalar.activation(
                            out=of, in_=of, func=AF.Identity,
                            bias=g_sb[:, 1, kc : kc + 1], scale=g_sb[:, 0, kc : kc + 1])
                    if fin_sb is not None:
                        nc.scalar.activation(
                            out=of, in_=of, func=AF.Identity,
                            bias=fin_sb[:, 1, kc : kc + 1], scale=fin_sb[:, 0, kc : kc + 1])
                    nc.sync.dma_start(dview[:, kc, sl], of)

        def load_w(dram, lidx, tag):
            _, fin, fout = dram.shape
            wt = wpool.tile([P, fin // P, fout], BF16, tag=tag)
            nc.sync.dma_start(wt, dram[lidx].rearrange("(c p) f -> p c f", p=P))
            return wt

        bias_sb = [None]

        def bias_mm(ps, row, mslice, tcslice_n):
            if bias_sb[0] is None:
                return
            nc.tensor.matmul(
                ps, lhsT=bias_sb[0][0:1, row, mslice], rhs=ones[0:1, 0:tcslice_n],
                start=False, stop=True, skip_group_check=True)

        def run_inject(inject, idx):
            if inject and idx in inject:
                for fn in inject[idx]:
                    fn()

        def emit_ffn_stripe(w1, w2, rows, src, tci, inject=None):
            """One stripe of an FFN: mms + silu + residual adds into x."""
            sl = slice(tci * NT, (tci + 1) * NT)
            acc = [psum.tile([P, NT], F32, tag="mm", name=f"acc{i}")
                   for i in range(DC)]
            for m in range(FC):
                run_inject(inject, m)
                ph = psum.tile([P, NT], F32, tag="mm", name="ph")
                for kc in range(DC):
                    nc.tensor.matmul(
                        ph, lhsT=w1[:, kc, m * P : (m + 1) * P], rhs=src[:, kc, sl],
                        start=(kc == 0), stop=(kc == DC - 1 and not bias_gen))
                bias_mm(ph, rows[0], slice(m * P, (m + 1) * P), NT)
                hb = hpool.tile([P, NT], BF16, tag="h1")
                nc.scalar.activation(out=hb, in_=ph, func=AF.Silu)
                for dcc in range(DC):
                    nc.tensor.matmul(
                        acc[dcc], lhsT=w2[:, m, dcc * P : (dcc + 1) * P], rhs=hb,
                        start=(m == 0), stop=(m == FC - 1 and not bias_gen),
                        skip_group_check=True)
            run_inject(inject, FC)
            for dcc in range(DC):
                bias_mm(acc[dcc], rows[1], slice(dcc * P, (dcc + 1) * P), NT)
                nc.vector.scalar_tensor_tensor(
                    out=x[:, dcc, sl], in0=acc[dcc], scalar=1.0,
                    in1=x[:, dcc, sl], op0=OP.bypass, op1=OP.add)

        # ---------------- attention sub-emitters ----------------

        def emit_qk_stripe(wq, wk, tci, inject=None):
            sl = slice(tci * NT, (tci + 1) * NT)
            for m in range(DC):
                run_inject(inject, m)
                pq = psum.tile([P, NT], F32, tag="mm", name="pq")
                for kc in range(DC):
                    nc.tensor.matmul(
                        pq, lhsT=wq[:, kc, m * P : (m + 1) * P], rhs=xh[:, kc, sl],
                        start=(kc == 0), stop=(kc == DC - 1 and not bias_gen))
                bias_mm(pq, BR_Q, slice(m * P, (m + 1) * P), NT)
                nc.scalar.copy(out=qT[:, m, sl], in_=pq)
                pk = psum.tile([P, NT], F32, tag="mm", name="pk")
                for kc in range(DC):
                    nc.tensor.matmul(
                        pk, lhsT=wk[:, kc, m * P : (m + 1) * P], rhs=xh[:, kc, sl],
                        start=(kc == 0), stop=(kc == DC - 1 and not bias_gen))
                bias_mm(pk, BR_K, slice(m * P, (m + 1) * P), NT)
                nc.scalar.copy(
                    out=kT[:, m, P + tci * NT : P + (tci + 1) * NT], in_=pk)

        def emit_v_blocks(wv, tbs, inject=None):
            for i, tb in enumerate(tbs):
                run_inject(inject, i)
                pv = psum.tile([P, NT], F32, tag="mm", name="pv")
                for kc in range(DC):
                    nc.tensor.matmul(
                        pv, lhsT=xh[:, kc, tb * P : (tb + 1) * P],
                        rhs=wv[:, kc, 0:D],
                        start=(kc == 0), stop=(kc == DC - 1 and not bias_gen))
                if bias_gen:
                    nc.tensor.matmul(
                        pv, lhsT=ones[0:1, 0:P], rhs=bias_sb[0][0:1, BR_V, 0:D],
                        start=False, stop=True, skip_group_check=True)
                nc.scalar.copy(out=vN[:, tb + 1, :], in_=pv)

        def emit_attn_core(qb, inject=None):
            band = bands["q0"] if qb == 0 else (
                bands["q7"] if qb == TB - 1 else bands["mid"])
            for hp in range(DC):
                run_inject(inject, hp)
                po = psum.tile([P, P], F32, tag="mm", name="po")
                den = psum.tile([P, P], F32, tag="mm", name="den")
                for hh in range(2):
                    pr = slice(hh * DH, (hh + 1) * DH)
                    st = psum.tile([P, 3, P], F32, tag="mm", name="st")
                    for c in range(3):
                        nc.tensor.matmul(
                            st[:, c, :],
                            lhsT=kT[pr, hp, qb * P + c * P : qb * P + (c + 1) * P],
                            rhs=qT[pr, hp, qb * P : (qb + 1) * P],
                            start=True, stop=True, skip_group_check=True)
                    at = atpool.tile([P, 3, P], BF16, tag="at", bufs=3)
                    nc.scalar.activation(out=at, in_=st, func=AF.Exp)
                    nc.vector.tensor_tensor(at, at, band, OP.mult)
                    h = hp * 2 + hh
                    hc = slice(h * DH, (h + 1) * DH)
                    for c in range(3):
                        nc.tensor.matmul(
                            po[pr, :], lhsT=vN[:, qb + c, hc], rhs=at[:, c, :],
                            start=(c == 0), stop=(c == 2), skip_group_check=True)
                    for c in range(3):
                        nc.tensor.matmul(
                            den[pr, :], lhsT=ones[:, 0:DH], rhs=at[:, c, :],
                            start=(c == 0), stop=(c == 2), skip_group_check=True)
                rden = atpool.tile([P, P], F32, tag="rden", bufs=2)
                nc.vector.reciprocal(out=rden, in_=den)
                nc.vector.tensor_tensor(
                    oT[:, hp, qb * P : (qb + 1) * P], po, rden, OP.mult)

        def emit_outproj_stripe(wo, tci, inject=None):
            sl = slice(tci * NT, (tci + 1) * NT)
            for m in range(DC):
                run_inject(inject, m)
                pp = psum.tile([P, NT], F32, tag="mm", name="pp")
                for kc in range(DC):
                    nc.tensor.matmul(
                        pp, lhsT=wo[:, kc, m * P : (m + 1) * P], rhs=oT[:, kc, sl],
                        start=(kc == 0), stop=(kc == DC - 1 and not bias_gen))
                bias_mm(pp, BR_O, slice(m * P, (m + 1) * P), NT)
                nc.vector.scalar_tensor_tensor(
                    out=x[:, m, sl], in0=pp, scalar=1.0, in1=x[:, m, sl],
                    op0=OP.bypass, op1=OP.add)

        # ---------------- conv sub-emitters ----------------

        def emit_conv(l, p1, p2, dwt16, dwb_sb, gn_ps, inject=None):
            """pw1+GLU+dwconv with GN sums accumulated in-loop."""
            gs, gq = gn_ps
            for m in range(CC):
                run_inject(inject, m)
                cp = cpool.tile([P, KK - 1 + T + 1], BF16, tag="cp", bufs=2)
                co = cpool.tile([P, KK - 1 + T + 1], BF16, tag="co", bufs=2)
                nc.vector.memset(cp[:, 0 : KK // 2], 0.0)
                nc.vector.memset(cp[:, KK // 2 + T :], 0.0)
                # diag strips for this channel chunk: one broadcast multiply
                strip = cpool.tile([P, KK, P], BF16, tag="strip", bufs=2)
                nc.vector.tensor_tensor(
                    strip,
                    ident.unsqueeze(1).to_broadcast((P, KK, P)),
                    dwt16[:, m, :].unsqueeze(2).to_broadcast((P, KK, P)),
                    OP.mult)
                for tci in range(TC):
                    sl = slice(tci * NT, (tci + 1) * NT)
                    pb = psum.tile([P, NT], F32, tag="mm", name="pb")
                    for kc in range(DC):
                        nc.tensor.matmul(
                            pb, lhsT=p1[:, kc, EC + m * P : EC + (m + 1) * P],
                            rhs=xh[:, kc, sl],
                            start=(kc == 0), stop=(kc == DC - 1 and not bias_gen))
                    bias_mm(pb, BR_P1, slice(EC + m * P, EC + (m + 1) * P), NT)
                    tb_ = tpool.tile([P, NT], BF16, tag="th")
                    nc.scalar.activation(out=tb_, in_=pb, func=AF.Tanh, scale=0.5)
                    pa = psum.tile([P, NT], F32, tag="mm", name="pa")
                    for kc in range(DC):
                        nc.tensor.matmul(
                            pa, lhsT=p1[:, kc, m * P : (m + 1) * P],
                            rhs=xh[:, kc, sl],
                            start=(kc == 0), stop=(kc == DC - 1 and not bias_gen))
                    bias_mm(pa, BR_P1, slice(m * P, (m + 1) * P), NT)
                    nc.vector.scalar_tensor_tensor(
                        out=cp[:, KK // 2 + tci * NT : KK // 2 + (tci + 1) * NT],
                        in0=tb_, scalar=1.0, in1=pa, op0=OP.add, op1=OP.mult)
                nc.vector.tensor_copy(out=co[:, 0 : KK - 1 + T],
                                      in_=cp[:, 1 : KK + T])
                for tci in range(TC):
                    pc = psum.tile([P, NT], F32, tag="mm", name="pc")
                    for kk in range(KK):
                        rhs = (cp[:, kk + tci * NT : kk + tci * NT + NT]
                               if kk % 2 == 0 else
                               co[:, kk - 1 + tci * NT : kk - 1 + tci * NT + NT])
                        nc.tensor.matmul(pc, lhsT=strip[:, kk, :], rhs=rhs,
                                         start=(kk == 0), stop=(kk == KK - 1),
                                         skip_group_check=True)
                    csl = c2[:, m, tci * NT : (tci + 1) * NT]
                    if dwb_sb is not None:
                        nc.scalar.activation(out=csl, in_=pc, func=AF.Identity,
                                             bias=dwb_sb[:, m : m + 1])
                    else:
                        nc.scalar.copy(out=csl, in_=pc)
                # GN sums for this chunk (accumulate across m and tci)
                c2sq = spool.tile([P, T], BF16, tag="gnsq", bufs=2)
                nc.scalar.activation(out=c2sq, in_=c2[:, m], func=AF.Square)
                for tci in range(TC):
                    sl = slice(tci * NT, (tci + 1) * NT)
                    nc.tensor.matmul(gs, lhsT=ones[:, 0:P], rhs=c2[:, m, sl],
                                     start=(m == 0 and tci == 0),
                                     stop=(m == CC - 1 and tci == TC - 1),
                                     skip_group_check=True)
                    nc.tensor.matmul(gq, lhsT=ones[:, 0:P], rhs=c2sq[:, sl],
                                     start=(m == 0 and tci == 0),
                                     stop=(m == CC - 1 and tci == TC - 1),
                                     skip_group_check=True)

        def emit_gn_finalize(l, gn_ps):
            """GroupNorm scalar chain -> per-channel affine (a_t, b_t)."""
            gs_ps, gq_ps = gn_ps
            rs = smpool.tile([P, 1], F32, tag="gs")
            rq = smpool.tile([P, 1], F32, tag="gq")
            nc.vector.tensor_reduce(out=rs, in_=gs_ps,
                                    axis=mybir.AxisListType.X, op=OP.add)
            nc.vector.tensor_reduce(out=rq, in_=gq_ps,
                                    axis=mybir.AxisListType.X, op=OP.add)
            mg = smpool.tile([P, 1], F32, tag="mg")
            nc.vector.tensor_scalar_mul(out=mg, in0=rs, scalar1=1.0 / (EC * T))
            msqg = smpool.tile([P, 1], F32, tag="msqg")
            nc.vector.tensor_tensor(msqg, mg, mg, OP.mult)
            varg = smpool.tile([P, 1], F32, tag="varg")
            nc.vector.scalar_tensor_tensor(
                out=varg, in0=rq, scalar=1.0 / (EC * T), in1=msqg,
                op0=OP.mult, op1=OP.subtract)
            nc.vector.tensor_scalar_add(out=varg, in0=varg, scalar1=EPS)
            rg = smpool.tile([P, 1], F32, tag="rg")
            ln_rstd(varg, rg, niter=14)
            gaff = spool.tile([P, 2, CC], F32, tag="gaff", bufs=2)
            nc.sync.dma_start(gaff, w_gn[l].rearrange("g (c p) -> p g c", p=P))
            a_t = spool.tile([P, CC], F32, tag="a_t", bufs=2)
            nc.vector.tensor_scalar_mul(out=a_t, in0=gaff[:, 0], scalar1=rg)
            mneg = smpool.tile([P, 1], F32, tag="mneg")
            nc.vector.tensor_scalar_mul(out=mneg, in0=mg, scalar1=-1.0)
            b_t = spool.tile([P, CC], F32, tag="b_t", bufs=2)
            nc.vector.scalar_tensor_tensor(
                out=b_t, in0=a_t, scalar=mneg, in1=gaff[:, 1],
                op0=OP.mult, op1=OP.add)
            return a_t, b_t

        def emit_pw2(p2, a_t, b_t, tci, inject=None):
            """GN affine + SiLU fused on ACT per chunk; m-outer pw2 acc."""
            sl = slice(tci * NT, (tci + 1) * NT)
            acc = [psum.tile([P, NT], F32, tag="mm", name=f"cacc{i}")
                   for i in range(DC)]
            for m in range(CC):
                run_inject(inject, m)
                if tci == 0:
                    nc.scalar.activation(
                        out=c2[:, m], in_=c2[:, m], func=AF.Silu,
                        bias=b_t[:, m : m + 1], scale=a_t[:, m : m + 1])
                for dcc in range(DC):
                    nc.tensor.matmul(
                        acc[dcc], lhsT=p2[:, m, dcc * P : (dcc + 1) * P],
                        rhs=c2[:, m, sl],
                        start=(m == 0), stop=(m == CC - 1 and not bias_gen),
                        skip_group_check=True)
            run_inject(inject, CC)
            for dcc in range(DC):
                bias_mm(acc[dcc], BR_P2, slice(dcc * P, (dcc + 1) * P), NT)
                nc.vector.scalar_tensor_tensor(
                    out=x[:, dcc, sl], in0=acc[dcc], scalar=1.0,
                    in1=x[:, dcc, sl], op0=OP.bypass, op1=OP.add)

        # ================= layer driver =================

        # initial LN for layer 0 (x raw -> xh)
        st0 = ln_sq(x, 0, 0, 0)
        ln_sums(st0, 0)
        ln_var(st0, 0)
        ln_sq(x, 0, 0, 1, st0)
        ln_sums(st0, 1)
        ln_var(st0, 1)
        ln_norm(st0, 0, xh)
        ln_norm(st0, 1, xh)

        # pending LN-norm emitters carried across phases
        for l in range(LAYERS):
            if bias_gen:
                bt = wpool.tile([1, 10, 2 * EC], BF16, tag="bias")
                nc.sync.dma_start(bt, w_bias[l])
                bias_sb[0] = bt

            w1 = load_w(w_f1a, l, "w1")
            w2 = load_w(w_f1b, l, "w2")
            src1 = xh if l == 0 else x

            # ===== FFN1 (+ attn-LN pipelined) =====
            stA = [None]
            emit_ffn_stripe(w1, w2, (BR_F1B1, BR_F1B2), src1, 0)
            stA[0] = ln_sq(x, l, 1, 0)
            emit_ffn_stripe(
                w1, w2, (BR_F1B1, BR_F1B2), src1, 1,
                inject={4: [lambda: ln_sums(stA[0], 0)],
                        6: [lambda: ln_var(stA[0], 0)],
                        10: [lambda: ln_norm(stA[0], 0, xh)]})
            ln_sq(x, l, 1, 1, stA[0])

            # ===== attention =====
            wq = load_w(w_q, l, "wq")
            wk = load_w(w_kk, l, "wk")
            wv = load_w(w_v, l, "wv")
            wo = load_w(w_o, l, "wo")
            emit_qk_stripe(wq, wk, 0,
                           inject={1: [lambda: ln_sums(stA[0], 1)],
                                   2: [lambda: ln_var(stA[0], 1)],
                                   3: [lambda: ln_norm(stA[0], 1, xh)]})
            emit_qk_stripe(wq, wk, 1)
            emit_v_blocks(wv, range(TB))
            stC = [None]

            def core_tail_0():
                emit_outproj_stripe(wo, 0)
                stC[0] = ln_sq(x, l, 2, 0)

            for qb in range(4):
                emit_attn_core(qb)
            core_tail_0()
            emit_attn_core(4, inject={2: [lambda: ln_sums(stC[0], 0)]})
            for qb in range(5, TB):
                emit_attn_core(qb)
            emit_outproj_stripe(wo, 1, inject={2: [lambda: ln_var(stC[0], 0)]})
            ln_sq(x, l, 2, 1, stC[0])
            ln_norm(stC[0], 0, xh)

            # ===== conv module =====
            p1 = load_w(w_p1, l, "w1")
            p2 = load_w(w_p2, l, "w2")
            dwt = wpool.tile([P, CC, KK], F32, tag="dw")
            nc.sync.dma_start(dwt, w_dw[l])
            dwt16 = wpool.tile([P, CC, KK], BF16, tag="dw16")
            nc.vector.tensor_copy(out=dwt16, in_=dwt)
            dwb_sb = None
            if dwb_gen:
                dwb_sb = wpool.tile([P, CC], F32, tag="dwb")
                nc.sync.dma_start(dwb_sb, w_dwb[l])
            gs = psum.tile([P, NT], F32, tag="mm", name="gn_s")
            gq = psum.tile([P, NT], F32, tag="mm", name="gn_q")

            # conv-LN stripe-1 stats emitted just before the pw1/dw stretch:
            # pw1 m=0 reads xh stripe 1, so its normalize must precede it
            def conv_inject():
                ln_sums(stC[0], 1)
                ln_var(stC[0], 1)
                ln_norm(stC[0], 1, xh)

            emit_conv(l, p1, p2, dwt16, dwb_sb, (gs, gq),
                      inject={0: [conv_inject]})

            a_t, b_t = emit_gn_finalize(l, (gs, gq))
            st2 = [None]
            emit_pw2(p2, a_t, b_t, 0)
            st2[0] = ln_sq(x, l, 3, 0)
            emit_pw2(p2, a_t, b_t, 1,
                     inject={2: [lambda: ln_sums(st2[0], 0)],
                             4: [lambda: ln_var(st2[0], 0)],
                             6: [lambda: ln_norm(st2[0], 0, xh)]})
            ln_sq(x, l, 3, 1, st2[0])

            # ===== FFN2 (+ blk-LN pipelined) =====
            w1b = load_w(w_f2a, l, "w1")
            w2b = load_w(w_f2b, l, "w2")
            stB = [None]
            emit_ffn_stripe(
                w1b, w2b, (BR_F2B1, BR_F2B2), xh, 0,
                inject={2: [lambda: ln_sums(st2[0], 1)],
                        4: [lambda: ln_var(st2[0], 1)],
                        8: [lambda: ln_norm(st2[0], 1, xh)]})
            stB[0] = ln_sq(x, l, 4, 0)
            emit_ffn_stripe(
                w1b, w2b, (BR_F2B1, BR_F2B2), xh, 1,
                inject={4: [lambda: ln_sums(stB[0], 0)],
                        6: [lambda: ln_var(stB[0], 0)]})
            ln_sq(x, l, 4, 1, stB[0])

            # ===== per-block LN =====
            last = l == LAYERS - 1
            fin_sb = None
            if last and w_fin is not None:
                fin_sb = spool.tile([P, 2, DC], F32, tag="fin_sb")
                nc.sync.dma_start(
                    fin_sb, w_fin.rearrange("g (c p) -> p g c", p=P))
            dview = out_d.rearrange("(c p) t -> p c t", p=P) if last else None
            ln_norm(stB[0], 0, None if last else x,
                    out_stream=dview, fin_sb=fin_sb)
            ln_sums(stB[0], 1)
            ln_var(stB[0], 1)
            ln_norm(stB[0], 1, None if last else x,
                    out_stream=dview, fin_sb=fin_sb)
            # stripe-1 stat mms for blk-LN run between phases; next layer's
            # FFN1 stripe-0 mms only need x stripe 0 (already normalized)

        if LAYERS == 0:
            with tc.tile_pool(name="outp0", bufs=3) as op_:
                dview = out_d.rearrange("(c p) t -> p c t", p=P)
                for kc in range(DC):
                    for tci in range(TC):
                        sl = slice(tci * NT, (tci + 1) * NT)
                        of = op_.tile([P, NT], F32, tag="of")
                        nc.vector.tensor_copy(out=of, in_=x[:, kc, sl])
                        nc.sync.dma_start(dview[:, kc, sl], of)

    nc.finalize()
    return nc


_PROG_CACHE = {}


def _get_program(flags):
    key = tuple(sorted(flags.items())) + (LAYERS,)
    if key not in _PROG_CACHE:
        _PROG_CACHE[key] = build_program(flags)
    return _PROG_CACHE[key]


def kernel(**inputs):
    global LAST_RESULT
    f32 = lambda a: np.asarray(a, dtype=np.float32)
    bf = lambda a: np.ascontiguousarray(f32(a).astype(ml_dtypes.bfloat16))
    x = f32(inputs["x"])                       # [B, T, D]

    def triv(names_vals):
        return all(bool(np.all(f32(inputs[n]) == v)) for n, v in names_vals)

    ln_trivial = triv(
        [(f"{p}_ln_g", 1.0) for p in ("ffn1", "attn", "conv", "ffn2", "blk")]
        + [(f"{p}_ln_b", 0.0) for p in ("ffn1", "attn", "conv", "ffn2", "blk")])
    final_trivial = triv([("final_ln_g", 1.0), ("final_ln_b", 0.0)])
    bias_trivial = triv([(n, 0.0) for n in (
        "ffn1_b1", "ffn1_b2", "qkv_b", "outp_b", "pw1_b", "pw2_b",
        "ffn2_b1", "ffn2_b2")])
    dwb_trivial = triv([("dw_b", 0.0)])
    flags = dict(ln_trivial=ln_trivial, final_trivial=final_trivial,
                 bias_trivial=bias_trivial, dwb_trivial=dwb_trivial)

    nc = _get_program(flags)

    qkv = f32(inputs["qkv_w"])                # [L, D, 3D]
    dw = f32(inputs["dw_w"]).reshape(L, EC, KK) * 0.5
    dw = dw.reshape(L, CC, P, KK).transpose(0, 2, 1, 3)  # [L, P, CC, K]
    gn_aff = np.stack([f32(inputs["gn_g"]), f32(inputs["gn_b"])], axis=1)

    common = {
        "f1w1": bf(inputs["ffn1_w1"]),
        "f1w2": bf(f32(inputs["ffn1_w2"]) * 0.5),
        "f2w1": bf(inputs["ffn2_w1"]),
        "f2w2": bf(f32(inputs["ffn2_w2"]) * 0.5),
        "wq": bf(qkv[:, :, 0:D] * (DH ** -0.5)),
        "wk": bf(qkv[:, :, D : 2 * D]),
        "wv": bf(qkv[:, :, 2 * D : 3 * D]),
        "wo": bf(inputs["outp_w"]),
        "pw1": bf(inputs["pw1_w"]),
        "pw2": bf(inputs["pw2_w"]),
        "dw": np.ascontiguousarray(dw.astype(np.float32)),
        "gn_aff": np.ascontiguousarray(gn_aff.astype(np.float32)),
    }
    if not ln_trivial:
        rows = []
        for pfx in ("ffn1", "attn", "conv", "ffn2", "blk"):
            rows.append(f32(inputs[f"{pfx}_ln_g"]))
            rows.append(f32(inputs[f"{pfx}_ln_b"]))
        common["ln_gains"] = np.ascontiguousarray(
            np.stack(rows, axis=1).astype(np.float32))  # [L, 10, D]
    if not final_trivial:
        common["final_aff"] = np.ascontiguousarray(np.stack(
            [f32(inputs["final_ln_g"]), f32(inputs["final_ln_b"])]).astype(np.float32))
    if not bias_trivial:
        bias = np.zeros((L, 10, 2 * EC), np.float32)
        qb = f32(inputs["qkv_b"])
        bias[:, BR_F1B1, :FF] = f32(inputs["ffn1_b1"])
        bias[:, BR_F1B2, :D] = f32(inputs["ffn1_b2"]) * 0.5
        bias[:, BR_Q, :D] = qb[:, 0:D] * (DH ** -0.5)
        bias[:, BR_K, :D] = qb[:, D : 2 * D]
        bias[:, BR_V, :D] = qb[:, 2 * D : 3 * D]
        bias[:, BR_O, :D] = f32(inputs["outp_b"])
        bias[:, BR_P1, : 2 * EC] = f32(inputs["pw1_b"])
        bias[:, BR_P2, :D] = f32(inputs["pw2_b"])
        bias[:, BR_F2B1, :FF] = f32(inputs["ffn2_b1"])
        bias[:, BR_F2B2, :D] = f32(inputs["ffn2_b2"]) * 0.5
        common["biases"] = bf(bias)
    if not dwb_trivial:
        dwb = f32(inputs["dw_b"]).reshape(L, CC, P).transpose(0, 2, 1)
        common["dwb"] = np.ascontiguousarray(dwb.astype(np.float32))

    in_maps = []
    for c in range(B):
        m = dict(common)
        m["x_t"] = np.ascontiguousarray(x[c].T)   # [D, T] fp32
        in_maps.append(m)

    res = run_bass_kernel_spmd(
        nc, in_maps, core_ids=list(range(B)), trace=TRACE, **TRACE_KW)
    LAST_RESULT = res
    out = np.stack([r["out_t"].T for r in res.results]).astype(np.float32)
    return out


if __name__ == "__main__":
    rng = np.random.default_rng(0)
    ins = {"x": rng.standard_normal((B, T, D), dtype=np.float32)}
    print("use test.py")
